# revision 1
# baseline (speedup 1.0000x reference)
"""Trainium2 Bass kernel for Qwen-style GQA attention block (B=2,S=2048,H=16,KV=8,D=128).

Sharding (8 cores): batch(2) x si-stripes(2) x head-half(2).
  core c: b=c>>2, sh=(c>>1)&1, hh=c&1
  - Q proj + attention for 8 q-heads (hh half) on 8 causally-balanced si blocks (sh stripes)
  - K/V proj for 4 kv heads over full S (replicated across the 2 stripe cores)
  - pair AllGather of ctx^T between the two head-half cores, then column-split o_proj.
All matmuls bf16 with fp32 PSUM accumulation. Softmax without max-subtraction
(scores are O(1) after QK RMSNorm); denominator via an appended ones-column on V.
"""
import sys

sys.path.insert(0, '/opt/trn_rl_repo')

import numpy as np

import concourse.bass as bass
import concourse.tile as tile
from concourse import mybir
from concourse.vector_clock import ScopedClock, VectorClock

B, S, HID = 2, 2048, 2048
H, KV, D = 16, 8, 128
EPS = 1e-6
SCALE = D ** -0.5
NBLK = S // 128  # 16
# causally balanced si-block stripes: sum(i+1) = 68 for both
MYBLKS = [[0, 2, 4, 6, 9, 11, 13, 15], [1, 3, 5, 7, 8, 10, 12, 14]]

F32 = mybir.dt.float32
BF16 = mybir.dt.bfloat16
AF = mybir.ActivationFunctionType


# ---------------------------------------------------------------------------
# Workarounds: this walrus supports only ONE sync-wait per instruction.
def _patched_drain_and_barrier(self, tick_clock, wait_clock):
    gc = tick_clock.global_clock
    vec = list(gc)
    nz = [i for i, v in enumerate(vec) if v > 0] or [0]
    for i in nz:
        cvec = [vec[j] if j == i else 0 for j in range(len(vec))]
        inst = self.nc.sync.drain()
        wait_clock.add_sem_waits(inst.ins, ScopedClock({None: VectorClock(cvec)}))
    self.nc.all_engine_barrier()
    assert self.sems is not None
    popped = self.nc._tile_sem_poison_stack.pop()
    assert popped is self._sem_poison
    self.nc.clear_and_free_semaphores(list(self.sems.allocated().values()))
    self.nc.all_engine_barrier()


tile.TileContext._drain_and_barrier = _patched_drain_and_barrier


def split_multi_waits(nc):
    for fn in nc.m.functions:
        for blk in fn.blocks:
            insts = list(blk.instructions)
            out = []
            changed = False
            for inst in insts:
                si = inst.sync_info
                if si is not None and len(si.on_wait) > 1:
                    waits = list(si.on_wait)
                    for k, w in enumerate(waits[:-1]):
                        out.append(mybir.InstNoOp(
                            name=f"{inst.name}.w{k}", engine=inst.engine,
                            sync_info=mybir.SyncInfo(on_wait=[w], on_update=[]),
                            text_hint="waitsplit"))
                    si.on_wait = [waits[-1]]
                    changed = True
                out.append(inst)
            if changed:
                blk.instructions[:] = out


# ---------------------------------------------------------------------------
def build_kernel():
    nc = bass.Bass(trn_type='TRN2')
    hT = nc.dram_tensor('hT', [HID, S], F32, kind='ExternalInput')
    qwT = nc.dram_tensor('qwT', [HID, 1024], F32, kind='ExternalInput')
    kwT = nc.dram_tensor('kwT', [HID, 512], F32, kind='ExternalInput')
    vwT = nc.dram_tensor('vwT', [HID, 512], F32, kind='ExternalInput')
    owT = nc.dram_tensor('owT', [2048, 1024], F32, kind='ExternalInput')
    # host-fused rope tables (cos/sin x norm-weight halves), [rows, 4, 64]
    qtab = nc.dram_tensor('qtab', [1024, 4, 64], F32, kind='ExternalInput')
    ktab = nc.dram_tensor('ktab', [S, 4, 64], F32, kind='ExternalInput')
    tri = nc.dram_tensor('tri', [128, 128], F32, kind='ExternalInput')
    iden = nc.dram_tensor('iden', [128, 128], F32, kind='ExternalInput')
    out_e = nc.dram_tensor('out', [1024, 1024], F32, kind='ExternalOutput')

    from contextlib import ExitStack
    with ExitStack() as ctx:
        tc = ctx.enter_context(tile.TileContext(nc))
        pool = lambda name, bufs, **kw: ctx.enter_context(
            tc.tile_pool(name=name, bufs=bufs, **kw))
        p_wq = pool('wq', 16)
        p_wk = pool('wk', 16)
        p_wv = pool('wv', 16)
        p_ht = pool('ht', 16)
        p_qt = pool('qt', 8)
        p_kt = pool('kt', 4)
        p_va = pool('va', 4)
        p_ctm = pool('ctm', 8)
        p_c = pool('const', 1)
        p_w = pool('work', 2)
        p_s = pool('small', 4)
        p_scl = pool('scl', 1)
        p_exp = pool('expb', 4)
        p_out = pool('outb', 2)
        ps_a = pool('psA', 2, space='PSUM')
        ps_s = pool('psS', 2, space='PSUM')
        ps_c = pool('psC', 2, space='PSUM')
        ps_t = pool('psT', 2, space='PSUM')
        p_d = pool('dram', 1, space='DRAM')
        if True:
            # ---- constants / weights (cast to bf16 on load) ----
            tri_s = p_c.tile([128, 128], BF16)
            nc.gpsimd.dma_start(tri_s[:], tri[:])
            iden_s = p_c.tile([128, 128], BF16)
            nc.gpsimd.dma_start(iden_s[:], iden[:])
            qtab_s = p_c.tile([128, 8, 4, 64], BF16)
            nc.gpsimd.dma_start(qtab_s[:], qtab.rearrange('(n p) t d -> p n t d', p=128))
            ktab_s = p_c.tile([128, 16, 4, 64], BF16)
            nc.gpsimd.dma_start(ktab_s[:], ktab.rearrange('(n p) t d -> p n t d', p=128))

            wq_s = [p_wq.tile([128, 1024], BF16, tag='wq', name='wq') for _ in range(16)]
            wk_s = [p_wk.tile([128, 512], BF16, tag='wk', name='wk') for _ in range(16)]
            wv_s = [p_wv.tile([128, 512], BF16, tag='wv', name='wv') for _ in range(16)]
            for ch in range(16):
                r = bass.ts(ch, 128)
                nc.gpsimd.dma_start(wq_s[ch][:], qwT[r, :])
                nc.gpsimd.dma_start(wk_s[ch][:], kwT[r, :])
                nc.gpsimd.dma_start(wv_s[ch][:], vwT[r, :])

            # persistent activation tiles
            QT = [p_qt.tile([128, 1024], BF16, tag='qt', name='qtl') for _ in range(8)]
            KT = [p_kt.tile([128, 2048], BF16, tag='kt', name='ktl') for _ in range(4)]
            VA = [p_va.tile([128, 16, 132], BF16, tag='va', name='va') for _ in range(4)]
            sclK = p_scl.tile([128, 16, 4], F32)   # SCALE * rstd_k per (sj_blk, kv)
            ctm = [p_ctm.tile([128, 1024], BF16, tag='ctm', name='ctm') for _ in range(8)]

            for kvh in range(4):  # ones column for the softmax denominator
                nc.gpsimd.memset(VA[kvh][:, :, 128:129], 1.0)

            bounds = [max(MYBLKS[0][bi], MYBLKS[1][bi]) for bi in range(8)]
            # per-core diagonal masks: dmask[bi][j] for j in {bounds[bi]-1, bounds[bi]}
            # encoded via a single input: dm [8, 2, 128, 128]
            dm = nc.dram_tensor('dm', [8, 2, 128, 128], F32, kind='ExternalInput')
            dm_s = p_c.tile([128, 8, 2, 128], BF16)
            nc.gpsimd.dma_start(dm_s[:], dm.rearrange('n t p d -> p n t d'))

            # ---- projections, two passes over s-halves ----
            for ph in range(2):
                ht_t = [p_ht.tile([128, 1024], BF16, tag='ht', name='ht') for _ in range(16)]
                for ch in range(16):
                    nc.gpsimd.dma_start(
                        ht_t[ch][:], hT[bass.ts(ch, 128), bass.ts(ph, 1024)])
                for j in range(8):
                    sb = ph * 8 + j
                    sslice = bass.ts(j, 128)
                    # ---- V ----
                    psV = ps_a.tile([128, 512], F32, tag='psA', name='psA')
                    for ch in range(16):
                        nc.tensor.matmul(psV[:], ht_t[ch][:, sslice], wv_s[ch][:],
                                         start=(ch == 0), stop=(ch == 15))
                    for kvh in range(4):
                        nc.scalar.copy(VA[kvh][:, sb, 0:128], psV[:, bass.ts(kvh, 128)])
                    # ---- K ----
                    psK = ps_a.tile([128, 512], F32, tag='psA', name='psA')
                    for ch in range(16):
                        nc.tensor.matmul(psK[:], ht_t[ch][:, sslice], wk_s[ch][:],
                                         start=(ch == 0), stop=(ch == 15))
                    kcp = p_w.tile([128, 512], F32, tag='kcp', name='kcp')
                    nc.scalar.copy(kcp[:], psK[:])
                    scr = p_w.tile([128, 512], F32, tag='scr', name='scr')
                    ss = p_s.tile([128, 4], F32, tag='ss', name='ss')
                    for kvh in range(4):
                        nc.scalar.activation(scr[:, bass.ts(kvh, 128)],
                                             kcp[:, bass.ts(kvh, 128)], AF.Square,
                                             accum_out=ss[:, kvh:kvh + 1])
                    nc.vector.tensor_scalar_add(ss[:], ss[:], float(EPS * D))
                    std = p_s.tile([128, 4], F32, tag='std', name='std')
                    nc.scalar.activation(std[:], ss[:], AF.Sqrt, scale=1.0 / D, bias=0.0)
                    rstd = p_s.tile([128, 4], F32, tag='rstd', name='rstd')
                    nc.vector.reciprocal(rstd[:], std[:])
                    nc.vector.tensor_scalar_mul(sclK[:, sb, :], rstd[:], SCALE)
                    # rope on raw K (w folded into ktab; rstd folded into exp scale)
                    kro = p_w.tile([128, 4, 128], BF16, tag='kro', name='kro')
                    lo = kcp[:].rearrange('p (t d) -> p t d', t=4)[:, :, 0:64]
                    hi = kcp[:].rearrange('p (t d) -> p t d', t=4)[:, :, 64:128]
                    tA = ktab_s[:, sb, :, :][:, 0:1, :]
                    tB = ktab_s[:, sb, :, :][:, 1:2, :]
                    tC = ktab_s[:, sb, :, :][:, 2:3, :]
                    tD = ktab_s[:, sb, :, :][:, 3:4, :]
                    t1 = p_w.tile([128, 4, 64], F32, tag='t1', name='t1')
                    t2 = p_w.tile([128, 4, 64], F32, tag='t2', name='t2')
                    mul_b(nc, t1[:], lo, tA)
                    mul_b(nc, t2[:], hi, tB)
                    nc.vector.tensor_sub(kro[:, :, 0:64], t1[:], t2[:])
                    mul_b(nc, t1[:], hi, tC)
                    mul_b(nc, t2[:], lo, tD)
                    nc.vector.tensor_add(kro[:, :, 64:128], t1[:], t2[:])
                    for kvh in range(4):  # transpose to KT
                        pst = ps_t.tile([128, 128], BF16, tag='psT', name='psT')
                        nc.tensor.transpose(pst[:], kro[:, kvh, :], iden_s[:])
                        nc.scalar.copy(KT[kvh][:, bass.ts(sb, 128)], pst[:])
                del ht_t

            # ---- Q projection from host-gathered hTq (my si rows, local order) ----
            hTq = nc.dram_tensor('hTq', [HID, 1024], F32, kind='ExternalInput')
            htq_t = [p_ht.tile([128, 1024], BF16, tag='ht', name='ht') for _ in range(16)]
            for ch in range(16):
                nc.gpsimd.dma_start(htq_t[ch][:], hTq[bass.ts(ch, 128), :])
            for bi in range(8):
                sslice = bass.ts(bi, 128)
                for qg in range(2):
                    psQ = ps_a.tile([128, 512], F32, tag='psA', name='psA')
                    for ch in range(16):
                        nc.tensor.matmul(psQ[:], htq_t[ch][:, sslice],
                                         wq_s[ch][:, bass.ts(qg, 512)],
                                         start=(ch == 0), stop=(ch == 15))
                    qcp = p_w.tile([128, 512], F32, tag='kcp', name='qcp')
                    nc.scalar.copy(qcp[:], psQ[:])
                    scr = p_w.tile([128, 512], F32, tag='scr', name='scr')
                    ss = p_s.tile([128, 4], F32, tag='ss', name='ss')
                    for hq in range(4):
                        nc.scalar.activation(scr[:, bass.ts(hq, 128)],
                                             qcp[:, bass.ts(hq, 128)], AF.Square,
                                             accum_out=ss[:, hq:hq + 1])
                    nc.vector.tensor_scalar_add(ss[:], ss[:], float(EPS * D))
                    std = p_s.tile([128, 4], F32, tag='std', name='std')
                    nc.scalar.activation(std[:], ss[:], AF.Sqrt, scale=1.0 / D, bias=0.0)
                    rstd = p_s.tile([128, 4], F32, tag='rstd', name='rstd')
                    nc.vector.reciprocal(rstd[:], std[:])
                    qro = p_w.tile([128, 4, 128], BF16, tag='kro', name='kro')
                    lo = qcp[:].rearrange('p (t d) -> p t d', t=4)[:, :, 0:64]
                    hi = qcp[:].rearrange('p (t d) -> p t d', t=4)[:, :, 64:128]
                    tA = qtab_s[:, bi, :, :][:, 0:1, :]
                    tB = qtab_s[:, bi, :, :][:, 1:2, :]
                    tC = qtab_s[:, bi, :, :][:, 2:3, :]
                    tD = qtab_s[:, bi, :, :][:, 3:4, :]
                    t1 = p_w.tile([128, 4, 64], F32, tag='t1', name='t1')
                    t2 = p_w.tile([128, 4, 64], F32, tag='t2', name='t2')
                    mul_b(nc, t1[:], lo, tA)
                    mul_b(nc, t2[:], hi, tB)
                    nc.vector.tensor_sub(qro[:, :, 0:64], t1[:], t2[:])
                    mul_b(nc, t1[:], hi, tC)
                    mul_b(nc, t2[:], lo, tD)
                    nc.vector.tensor_add(qro[:, :, 64:128], t1[:], t2[:])
                    qn = p_w.tile([128, 4, 128], BF16, tag='qn', name='qn')
                    for hq in range(4):
                        nc.vector.tensor_scalar_mul(qn[:, hq, :], qro[:, hq, :],
                                                    rstd[:, hq:hq + 1])
                        pst = ps_t.tile([128, 128], BF16, tag='psT', name='psT')
                        nc.tensor.transpose(pst[:], qn[:, hq, :], iden_s[:])
                        nc.scalar.copy(QT[qg * 4 + hq][:, bass.ts(bi, 128)], pst[:])

            # ---- attention ----
            for h in range(8):
                kvh = h // 2
                for bi in range(8):
                    gi = bounds[bi]
                    psC = ps_c.tile([128, 132], F32, tag='psC', name='psC')
                    for j in range(gi + 1):
                        psS = ps_s.tile([128, 128], F32, tag='psS', name='psS')
                        nc.tensor.matmul(psS[:], KT[kvh][:, bass.ts(j, 128)],
                                         QT[h][:, bass.ts(bi, 128)],
                                         start=True, stop=True)
                        ex = p_exp.tile([128, 128], BF16, tag='expb', name='expb')
                        nc.scalar.activation(ex[:], psS[:], AF.Exp,
                                             scale=sclK[:, j, kvh:kvh + 1])
                        if j >= gi - 1:  # possible diagonal/overhang: apply mask
                            nc.vector.tensor_mul(ex[:], ex[:], dm_s[:, bi, j - (gi - 1), :])
                        nc.tensor.matmul(psC[:, 0:129], ex[:], VA[kvh][:, j, 0:129],
                                         start=(j == 0), stop=(j == gi))
                    rd = p_s.tile([128, 1], F32, tag='rd', name='rd')
                    nc.vector.reciprocal(rd[:], psC[:, 128:129])
                    cn = p_w.tile([128, 128], BF16, tag='cn', name='cn')
                    nc.vector.tensor_scalar_mul(cn[:], psC[:, 0:128], rd[:])
                    pst = ps_t.tile([128, 128], BF16, tag='psT', name='psT')
                    nc.tensor.transpose(pst[:], cn[:], iden_s[:])
                    nc.scalar.copy(ctm[h][:, bass.ts(bi, 128)], pst[:])

            wo_s = [p_ht.tile([128, 1024], BF16, tag='ht', name='wo') for _ in range(16)]
            for ch in range(16):
                nc.gpsimd.dma_start(wo_s[ch][:], owT[bass.ts(ch, 128), :])

            # ---- pair AllGather of ctx^T ----
            cc_in = p_d.tile([1024, 1024], BF16)
            cc_out = p_d.tile([2048, 1024], BF16)
            for h in range(8):
                nc.sync.dma_start(cc_in[bass.ts(h, 128), :], ctm[h][:])
            nc.gpsimd.collective_compute(
                'AllGather', mybir.AluOpType.bypass,
                replica_groups=[[0, 1], [2, 3], [4, 5], [6, 7]],
                ins=[cc_in.opt()], outs=[cc_out.opt()])
            ctf = [p_wq.tile([128, 1024], BF16, tag='wq', name='ctf') for _ in range(16)]
            for ch in range(16):
                nc.sync.dma_start(ctf[ch][:], cc_out[bass.ts(ch, 128), :])

            # ---- o_proj (my ho half columns) ----
            for bi in range(8):
                for nt in range(2):
                    psO = ps_a.tile([128, 512], F32, tag='psA', name='psA')
                    for ch in range(16):
                        nc.tensor.matmul(psO[:], ctf[ch][:, bass.ts(bi, 128)],
                                         wo_s[ch][:, bass.ts(nt, 512)],
                                         start=(ch == 0), stop=(ch == 15))
                    ob = p_out.tile([128, 512], F32, tag='outb', name='outb')
                    nc.scalar.copy(ob[:], psO[:])
                    nc.sync.dma_start(out_e[bass.ts(bi, 128), bass.ts(nt, 512)], ob[:])

    split_multi_waits(nc)
    return nc


def mul_b(nc, out, a, b):
    """tensor_tensor multiply with free-dim broadcast of b over dim 1."""
    a2, b2 = bass.broadcast_tensor_aps(a, b)
    nc.vector.tensor_mul(out, a2, b2)


# ---------------------------------------------------------------------------
_NC_CACHE = None


def _get_nc():
    global _NC_CACHE
    if _NC_CACHE is None:
        _NC_CACHE = build_kernel()
    return _NC_CACHE


def kernel(hidden_states, cos, sin, q_w, k_w, v_w, o_w, q_norm_w, k_norm_w):
    from concourse.bass_utils import run_bass_kernel_spmd

    hidden_states = np.asarray(hidden_states, np.float32)
    cos = np.asarray(cos, np.float32)
    sin = np.asarray(sin, np.float32)
    q_w = np.asarray(q_w, np.float32)
    k_w = np.asarray(k_w, np.float32)
    v_w = np.asarray(v_w, np.float32)
    o_w = np.asarray(o_w, np.float32)
    q_norm_w = np.asarray(q_norm_w, np.float32)
    k_norm_w = np.asarray(k_norm_w, np.float32)

    tri_np = np.triu(np.ones((128, 128), np.float32))  # [sj,si]: valid sj<=si
    iden_np = np.eye(128, dtype=np.float32)

    def rope_tabs(c, s_, w):
        # tables [rows, 4, 64]: A=c_lo*w_lo, B=s_lo*w_hi, C=c_lo*w_hi, D=s_lo*w_lo
        cl, sl = c[:, 0:64], s_[:, 0:64]
        wl, wh = w[0:64], w[64:128]
        return np.stack([cl * wl, sl * wh, cl * wh, sl * wl], axis=1).astype(np.float32)

    bounds = [max(MYBLKS[0][bi], MYBLKS[1][bi]) for bi in range(8)]

    in_maps = []
    for c in range(8):
        b, sh, hh = c >> 2, (c >> 1) & 1, c & 1
        blks = MYBLKS[sh]
        rows = np.concatenate([np.arange(g * 128, (g + 1) * 128) for g in blks])
        hT = np.ascontiguousarray(hidden_states[b].T)
        hTq = np.ascontiguousarray(hidden_states[b][rows].T)
        qwT = np.ascontiguousarray(q_w[hh * 1024:(hh + 1) * 1024].T)
        kwT = np.ascontiguousarray(k_w[hh * 512:(hh + 1) * 512].T)
        vwT = np.ascontiguousarray(v_w[hh * 512:(hh + 1) * 512].T)
        owT = np.ascontiguousarray(o_w[hh * 1024:(hh + 1) * 1024].T)
        qtab = rope_tabs(cos[b][rows], sin[b][rows], q_norm_w)
        ktab = rope_tabs(cos[b], sin[b], k_norm_w)
        # diagonal masks dm[bi, t]: t=0 -> sj block gi-1, t=1 -> sj block gi
        # my true causal diagonal is at block g=blks[bi] (<= bounds[bi]).
        dm = np.zeros((8, 2, 128, 128), np.float32)
        for bi in range(8):
            g, gb = blks[bi], bounds[bi]
            for t, j in enumerate((gb - 1, gb)):
                if j < 0:
                    continue
                if j < g:
                    dm[bi, t] = 1.0
                elif j == g:
                    dm[bi, t] = tri_np
                # j > g: stays 0 (block fully masked)
        in_maps.append(dict(
            hT=hT, hTq=hTq, qwT=qwT, kwT=kwT, vwT=vwT, owT=owT,
            qtab=qtab, ktab=ktab, tri=tri_np, iden=iden_np, dm=dm))

    nc = _get_nc()
    res = run_bass_kernel_spmd(nc, in_maps, core_ids=list(range(8)))

    out = np.zeros((B, S, HID), np.float32)
    for c in range(8):
        b, sh, hh = c >> 2, (c >> 1) & 1, c & 1
        o = res.results[c]['out']  # [1024, 1024]
        for bi, g in enumerate(MYBLKS[sh]):
            out[b, g * 128:(g + 1) * 128, hh * 1024:(hh + 1) * 1024] = \
                o[bi * 128:(bi + 1) * 128]
    return out


if __name__ == '__main__':
    sys.path.insert(0, '/root/problem')
    import reference
    inputs = {k: np.asarray(v) for k, v in reference.setup_inputs().items()}
    exp = np.asarray(reference.reference(**inputs))
    act = kernel(**inputs)
    err = np.abs(act - exp)
    rel = np.linalg.norm(act - exp) / np.linalg.norm(exp)
    print('Relative error:', rel, 'max abs err:', err.max())



# revision 45
# speedup vs baseline: 61605.7934x; 61605.7934x over previous
"""Trainium2 Bass kernel for Qwen-style GQA attention block (B=2,S=2048,H=16,KV=8,D=128).

Sharding (8 cores): batch(2) x si-stripes(2) x head-half(2).
  core c: b=c>>2, sh=(c>>1)&1, hh=c&1
v2 layout:
  - all big inputs host-cast to bf16, loaded via HWDGE
  - K/V projection computed only for the core's own 8 si blocks (from hTq),
    K-RMSNorm folded into K tiles, then pair AllGather across the stripe pair
    so each core gets full-S K^T / V for its 4 kv heads
  - attention: scores [sj, si] per j-block, exp batched 4 j-blocks per
    ACTIVATE (one PSUM bank), constant exp scale; software-pipelined emission
  - ctx AllGather split in two (heads 0-3 after they finish, then 4-7) to
    overlap with attention; column-split o_proj
"""
import sys

sys.path.insert(0, '/opt/trn_rl_repo')

import numpy as np
import ml_dtypes

import concourse.bass as bass
import concourse.tile as tile
from concourse import mybir
from concourse.vector_clock import ScopedClock, VectorClock

B, S, HID = 2, 2048, 2048
H, KV, D = 16, 8, 128
EPS = 1e-6
SCALE = D ** -0.5
NBLK = S // 128  # 16
# causally balanced si-block stripes: sum(i+1) = 68 for both
MYBLKS = [[0, 2, 4, 6, 9, 11, 13, 15], [1, 3, 5, 7, 8, 10, 12, 14]]
# global sj block -> column slot in the gathered KT/VA (stripe0 slots 0-7,
# stripe1 slots 8-15); identical mapping on every core
SLOT = {g: i for i, g in enumerate(MYBLKS[0] + MYBLKS[1])}
BOUNDS = [max(MYBLKS[0][bi], MYBLKS[1][bi]) for bi in range(8)]  # [1,3,..,15]

F32 = mybir.dt.float32
BF16 = mybir.dt.bfloat16
AF = mybir.ActivationFunctionType
BF = ml_dtypes.bfloat16


# ---------------------------------------------------------------------------
# Workarounds: this walrus supports only ONE sync-wait per instruction.
def _patched_drain_and_barrier(self, tick_clock, wait_clock):
    gc = tick_clock.global_clock
    vec = list(gc)
    nz = [i for i, v in enumerate(vec) if v > 0] or [0]
    for i in nz:
        cvec = [vec[j] if j == i else 0 for j in range(len(vec))]
        inst = self.nc.sync.drain()
        wait_clock.add_sem_waits(inst.ins, ScopedClock({None: VectorClock(cvec)}))
    self.nc.all_engine_barrier()
    assert self.sems is not None
    popped = self.nc._tile_sem_poison_stack.pop()
    assert popped is self._sem_poison
    self.nc.clear_and_free_semaphores(list(self.sems.allocated().values()))
    self.nc.all_engine_barrier()


tile.TileContext._drain_and_barrier = _patched_drain_and_barrier


def split_multi_waits(nc):
    for fn in nc.m.functions:
        for blk in fn.blocks:
            insts = list(blk.instructions)
            out = []
            changed = False
            for inst in insts:
                si = inst.sync_info
                if si is not None and len(si.on_wait) > 1:
                    waits = list(si.on_wait)
                    for k, w in enumerate(waits[:-1]):
                        out.append(mybir.InstNoOp(
                            name=f"{inst.name}.w{k}", engine=inst.engine,
                            sync_info=mybir.SyncInfo(on_wait=[w], on_update=[]),
                            text_hint="waitsplit"))
                    si.on_wait = [waits[-1]]
                    changed = True
                out.append(inst)
            if changed:
                blk.instructions[:] = out


def mul_b(nc, out, a, b):
    """tensor_tensor multiply with free-dim broadcast of b over dim 1."""
    a2, b2 = bass.broadcast_tensor_aps(a, b)
    nc.vector.tensor_mul(out, a2, b2)


# ---------------------------------------------------------------------------
def build_kernel():
    nc = bass.Bass(trn_type='TRN2')
    hTq = nc.dram_tensor('hTq', [HID, 1024], BF16, kind='ExternalInput')
    qwT = nc.dram_tensor('qwT', [HID, 1024], BF16, kind='ExternalInput')
    kwT = nc.dram_tensor('kwT', [HID, 512], BF16, kind='ExternalInput')
    vwT = nc.dram_tensor('vwT', [HID, 512], BF16, kind='ExternalInput')
    owT = nc.dram_tensor('owT', [2048, 1024], BF16, kind='ExternalInput')
    # host-fused rope tables (cos/sin x norm-weight halves), my rows, [1024,4,64]
    qtab = nc.dram_tensor('qtab', [1024, 4, 64], BF16, kind='ExternalInput')
    ktab = nc.dram_tensor('ktab', [1024, 4, 64], BF16, kind='ExternalInput')
    iden = nc.dram_tensor('iden', [128, 128], BF16, kind='ExternalInput')
    dm = nc.dram_tensor('dm', [8, 2, 128, 128], BF16, kind='ExternalInput')
    out_e = nc.dram_tensor('out', [1024, 1024], F32, kind='ExternalOutput')

    from contextlib import ExitStack
    with ExitStack() as ctx:
        tc = ctx.enter_context(tile.TileContext(nc))
        pool = lambda name, bufs, **kw: ctx.enter_context(
            tc.tile_pool(name=name, bufs=bufs, **kw))
        p_wq = pool('wq', 16)
        p_wk = pool('wk', 16)
        p_wv = pool('wv', 16)
        p_ht = pool('ht', 16)
        p_qt = pool('qt', 8)
        p_kt = pool('kt', 4)
        p_va = pool('va', 4)
        p_ctm = pool('ctm', 8)
        p_c = pool('const', 1)
        p_w = pool('work', 2)
        p_s = pool('small', 4)
        p_exp = pool('expb', 4)
        p_out = pool('outb', 2)
        p_d = pool('dram', 1, space='DRAM')
        # PSUM (8 banks) is phase-scoped: psT (transposes) lives through
        # proj+attention; psA only in proj; psS (2-bank score tiles) + psC
        # only in attention; psO only in o_proj.
        from contextlib import ExitStack as _ES
        ps_t = pool('psT', 2, space='PSUM')
        proj_ctx = _ES()
        ps_a = proj_ctx.enter_context(
            tc.tile_pool(name='psA', bufs=2, space='PSUM'))

        # ---- constants (small, first on the sync queue) ----
        iden_s = p_c.tile([128, 128], BF16)
        nc.sync.dma_start(iden_s[:], iden[:])

        # ---- weights/activations: htq on sync, wk/wv/qw on scalar queue ----
        wk_s = [p_wk.tile([128, 512], BF16, tag='wk', name='wk') for _ in range(16)]
        wv_s = [p_wv.tile([128, 512], BF16, tag='wv', name='wv') for _ in range(16)]
        ht_t = [p_ht.tile([128, 1024], BF16, tag='ht', name='ht') for _ in range(16)]
        for ch in range(16):
            r = bass.ts(ch, 128)
            nc.scalar.dma_start(wk_s[ch][:], kwT[r, :])
            nc.scalar.dma_start(wv_s[ch][:], vwT[r, :])
            nc.sync.dma_start(ht_t[ch][:], hTq[r, :])
        ktab_s = p_c.tile([128, 8, 4, 64], BF16)
        nc.sync.dma_start(ktab_s[:], ktab.rearrange('(n p) t d -> p n t d', p=128))
        qtab_s = p_c.tile([128, 8, 4, 64], BF16)
        nc.sync.dma_start(qtab_s[:], qtab.rearrange('(n p) t d -> p n t d', p=128))
        dm_s = p_c.tile([128, 8, 2, 128], BF16)
        nc.sync.dma_start(dm_s[:], dm.rearrange('n t p d -> p n t d'))
        wq_s = [p_wq.tile([128, 1024], BF16, tag='wq', name='wq') for _ in range(16)]
        for ch in range(16):
            nc.scalar.dma_start(wq_s[ch][:], qwT[bass.ts(ch, 128), :])

        # persistent activation tiles
        QT = [p_qt.tile([128, 1024], BF16, tag='qt', name='qtl') for _ in range(8)]
        KT = [p_kt.tile([128, 2048], BF16, tag='kt', name='ktl') for _ in range(4)]
        VA = [p_va.tile([128, 16, 132], BF16, tag='va', name='va') for _ in range(4)]
        ctm = [p_ctm.tile([128, 1024], BF16, tag='ctm', name='ctm') for _ in range(8)]

        # K/V pair-exchange DRAM buffers, split in two gathers (bi 0-3, 4-7)
        # so each stays on the low-latency Mesh path and fires early.
        # Per buffer: K region cols [0,2048) as (bi, kvh, sj), V region
        # cols [2048,4160) as (bi, kvh, 132)
        cc_kv_in = [p_d.tile([128, 4160], BF16, name=f'cc_kv_in{i}')
                    for i in range(2)]
        cc_kv_out = [p_d.tile([256, 4160], BF16, name=f'cc_kv_out{i}')
                     for i in range(2)]

        # ---- K/V projection for my 8 si blocks ----
        for bi in range(8):
            sslice = bass.ts(bi, 128)
            psV = ps_a.tile([128, 512], F32, tag='psA', name='psV')
            for ch in range(16):
                nc.tensor.matmul(psV[:], ht_t[ch][:, sslice], wv_s[ch][:],
                                 start=(ch == 0), stop=(ch == 15))
            vstg = p_w.tile([128, 4, 132], BF16, tag='vstg', name='vstg')
            nc.gpsimd.memset(vstg[:, :, 128:132], 0.0)
            nc.gpsimd.memset(vstg[:, :, 128:129], 1.0)
            for kvh in range(4):
                nc.vector.tensor_copy(vstg[:, kvh, 0:128],
                                      psV[:, bass.ts(kvh, 128)])
            nc.sync.dma_start(
                cc_kv_in[bi // 4][:, 2048 + (bi % 4) * 528:
                                  2048 + (bi % 4 + 1) * 528],
                vstg[:].rearrange('p k d -> p (k d)'))
            psK = ps_a.tile([128, 512], F32, tag='psA', name='psK')
            for ch in range(16):
                nc.tensor.matmul(psK[:], ht_t[ch][:, sslice], wk_s[ch][:],
                                 start=(ch == 0), stop=(ch == 15))
            kcp = p_w.tile([128, 512], F32, tag='kcp', name='kcp')
            nc.vector.tensor_copy(kcp[:], psK[:])
            scr = p_w.tile([128, 512], F32, tag='scr', name='scr')
            ss = p_s.tile([128, 4], F32, tag='ss', name='ss')
            for kvh in range(4):
                nc.scalar.activation(scr[:, bass.ts(kvh, 128)],
                                     kcp[:, bass.ts(kvh, 128)], AF.Square,
                                     accum_out=ss[:, kvh:kvh + 1])
            nc.vector.tensor_scalar_add(ss[:], ss[:], float(EPS * D))
            std = p_s.tile([128, 4], F32, tag='std', name='std')
            nc.scalar.activation(std[:], ss[:], AF.Sqrt, scale=1.0 / D, bias=0.0)
            rstd = p_s.tile([128, 4], F32, tag='rstd', name='rstd')
            nc.vector.reciprocal(rstd[:], std[:])
            # rope on raw K (norm weight folded into ktab)
            kro = p_w.tile([128, 4, 128], F32, tag='kro', name='kro')
            lo = kcp[:].rearrange('p (t d) -> p t d', t=4)[:, :, 0:64]
            hi = kcp[:].rearrange('p (t d) -> p t d', t=4)[:, :, 64:128]
            tA = ktab_s[:, bi, :, :][:, 0:1, :]
            tB = ktab_s[:, bi, :, :][:, 1:2, :]
            tC = ktab_s[:, bi, :, :][:, 2:3, :]
            tD = ktab_s[:, bi, :, :][:, 3:4, :]
            t1 = p_w.tile([128, 4, 64], F32, tag='t1', name='t1')
            t2 = p_w.tile([128, 4, 64], F32, tag='t2', name='t2')
            mul_b(nc, t1[:], lo, tA)
            mul_b(nc, t2[:], hi, tB)
            nc.vector.tensor_sub(kro[:, :, 0:64], t1[:], t2[:])
            mul_b(nc, t1[:], hi, tC)
            mul_b(nc, t2[:], lo, tD)
            nc.vector.tensor_add(kro[:, :, 64:128], t1[:], t2[:])
            kn = p_w.tile([128, 4, 128], BF16, tag='kn', name='kn')
            kstg = p_w.tile([128, 4, 128], BF16, tag='kstg', name='kstg')
            for kvh in range(4):   # fold K rmsnorm into K, transpose
                nc.vector.tensor_scalar_mul(kn[:, kvh, :], kro[:, kvh, :],
                                            rstd[:, kvh:kvh + 1])
                pst = ps_t.tile([128, 128], BF16, tag='psT', name='psT')
                nc.tensor.transpose(pst[:], kn[:, kvh, :], iden_s[:])
                nc.vector.tensor_copy(kstg[:, kvh, :], pst[:])
            nc.sync.dma_start(
                cc_kv_in[bi // 4][:, (bi % 4) * 512:(bi % 4 + 1) * 512],
                kstg[:].rearrange('p k d -> p (k d)'))
            if bi % 4 == 3:  # pair AllGather of this half of my K^T/V
                half = bi // 4
                nc.gpsimd.collective_compute(
                    'AllGather', mybir.AluOpType.bypass,
                    replica_groups=[[0, 1], [2, 3], [4, 5], [6, 7]],
                    ins=[cc_kv_in[half].opt()], outs=[cc_kv_out[half].opt()])

        # unpack both halves at the end (sync queue blocks harmlessly here)
        for half in range(2):
            kv_k = cc_kv_out[half][:, 0:2048].rearrange(
                'p (b k d) -> p b k d', k=4, d=128)
            kv_v = cc_kv_out[half][:, 2048:4160].rearrange(
                'p (b k d) -> p b k d', k=4, d=132)
            c0 = half * 512  # slots: stripe0 -> [c0,c0+512), stripe1 +1024
            for kvh in range(4):
                nc.sync.dma_start(
                    KT[kvh][:, c0:c0 + 512]
                    .rearrange('p (b d) -> p b d', d=128),
                    kv_k[0:128, :, kvh, :])
                nc.sync.dma_start(
                    KT[kvh][:, 1024 + c0:1024 + c0 + 512]
                    .rearrange('p (b d) -> p b d', d=128),
                    kv_k[128:256, :, kvh, :])
                nc.sync.dma_start(VA[kvh][:, half * 4:half * 4 + 4, :],
                                  kv_v[0:128, :, kvh, :])
                nc.sync.dma_start(VA[kvh][:, 8 + half * 4:8 + half * 4 + 4, :],
                                  kv_v[128:256, :, kvh, :])

        # ---- Q projection for my 8 si blocks (qg-outer: heads 0-3 first) ----
        for qg in range(2):
            for bi in range(8):
                sslice = bass.ts(bi, 128)
                psQ = ps_a.tile([128, 512], F32, tag='psA', name='psQ')
                for ch in range(16):
                    nc.tensor.matmul(psQ[:], ht_t[ch][:, sslice],
                                     wq_s[ch][:, bass.ts(qg, 512)],
                                     start=(ch == 0), stop=(ch == 15))
                qcp = p_w.tile([128, 512], F32, tag='kcp', name='qcp')
                nc.vector.tensor_copy(qcp[:], psQ[:])
                scr = p_w.tile([128, 512], F32, tag='scr', name='scr')
                ss = p_s.tile([128, 4], F32, tag='ss', name='ss')
                for hq in range(4):
                    nc.scalar.activation(scr[:, bass.ts(hq, 128)],
                                         qcp[:, bass.ts(hq, 128)], AF.Square,
                                         accum_out=ss[:, hq:hq + 1])
                nc.vector.tensor_scalar_add(ss[:], ss[:], float(EPS * D))
                std = p_s.tile([128, 4], F32, tag='std', name='std')
                nc.scalar.activation(std[:], ss[:], AF.Sqrt, scale=1.0 / D, bias=0.0)
                rstd = p_s.tile([128, 4], F32, tag='rstd', name='rstd')
                nc.vector.reciprocal(rstd[:], std[:])
                qro = p_w.tile([128, 4, 128], F32, tag='kro', name='qro')
                lo = qcp[:].rearrange('p (t d) -> p t d', t=4)[:, :, 0:64]
                hi = qcp[:].rearrange('p (t d) -> p t d', t=4)[:, :, 64:128]
                tA = qtab_s[:, bi, :, :][:, 0:1, :]
                tB = qtab_s[:, bi, :, :][:, 1:2, :]
                tC = qtab_s[:, bi, :, :][:, 2:3, :]
                tD = qtab_s[:, bi, :, :][:, 3:4, :]
                t1 = p_w.tile([128, 4, 64], F32, tag='t1', name='t1')
                t2 = p_w.tile([128, 4, 64], F32, tag='t2', name='t2')
                mul_b(nc, t1[:], lo, tA)
                mul_b(nc, t2[:], hi, tB)
                nc.vector.tensor_sub(qro[:, :, 0:64], t1[:], t2[:])
                mul_b(nc, t1[:], hi, tC)
                mul_b(nc, t2[:], lo, tD)
                nc.vector.tensor_add(qro[:, :, 64:128], t1[:], t2[:])
                qn = p_w.tile([128, 4, 128], BF16, tag='kn', name='qn')
                for hq in range(4):
                    nc.vector.tensor_scalar_mul(qn[:, hq, :], qro[:, hq, :],
                                                rstd[:, hq:hq + 1])
                    pst = ps_t.tile([128, 128], BF16, tag='psT', name='psT')
                    nc.tensor.transpose(pst[:], qn[:, hq, :], iden_s[:])
                    nc.vector.tensor_copy(QT[qg * 4 + hq][:, sslice], pst[:])
        del ht_t

        # o_proj weights (reuses the ht pool; loads during attention)
        wo_s = [p_ht.tile([128, 1024], BF16, tag='ht', name='wo') for _ in range(16)]
        for ch in range(16):
            nc.scalar.dma_start(wo_s[ch][:], owT[bass.ts(ch, 128), :])

        proj_ctx.close()   # free psA banks for the attention score tiles
        attn_ctx = _ES()
        ps_s = attn_ctx.enter_context(
            tc.tile_pool(name='psS', bufs=2, space='PSUM'))
        ps_c = attn_ctx.enter_context(
            tc.tile_pool(name='psC', bufs=2, space='PSUM'))

        # ---- attention, software-pipelined over (h, bi, bank) units ----
        # ctx AllGather split in four (2 heads each) so the collective stream
        # starts early and the last exchange is small.
        CTX_SPLIT = {1: (0, 2), 3: (2, 2), 5: (4, 2), 7: (6, 2)}
        cc_c_in = [p_d.tile([nh * 128, 1024], BF16, name=f'cc_c_in{h}')
                   for h, (h0, nh) in CTX_SPLIT.items()]
        cc_c_out = [p_d.tile([nh * 256, 1024], BF16, name=f'cc_c_out{h}')
                    for h, (h0, nh) in CTX_SPLIT.items()]
        ctf = [p_wq.tile([128, 1024], BF16, tag='wq', name='ctf')
               for _ in range(16)]
        # ctf order: gather k holds [g(2k),g(2k+1),g(2k+8),g(2k+9)];
        # matching o_w chunk for each ctf slot:
        CTF_WO = [0, 1, 8, 9, 2, 3, 10, 11, 4, 5, 12, 13, 6, 7, 14, 15]

        def emit_ctx_gather(ph):
            gi_, (h0, nh) = list(CTX_SPLIT.keys()).index(ph), CTX_SPLIT[ph]
            cin, cout = cc_c_in[gi_], cc_c_out[gi_]
            for i in range(nh):
                nc.sync.dma_start(cin[bass.ts(i, 128), :], ctm[h0 + i][:])
            nc.gpsimd.collective_compute(
                'AllGather', mybir.AluOpType.bypass,
                replica_groups=[[0, 2], [1, 3], [4, 6], [5, 7]],
                ins=[cin.opt()], outs=[cout.opt()])
            base = sum(2 * n for _, (_, n) in list(CTX_SPLIT.items())[:gi_])
            for i in range(2 * nh):
                nc.gpsimd.dma_start(ctf[base + i][:], cout[bass.ts(i, 128), :])


        units = []
        for h in range(8):
            for bi in range(8):
                gi = BOUNDS[bi]
                js = list(range(gi + 1))
                banks = [js[i:i + 8] for i in range(0, len(js), 8)]
                for k, bj in enumerate(banks):
                    units.append((h, bi, bj, k == len(banks) - 1))

        def emit_scores(u):
            h, bi, bj, last = u
            kvh = h // 2
            gi = BOUNDS[bi]
            psS = ps_s.tile([128, 8, 128], F32, tag='psS', name='psS')
            for t, j in enumerate(bj):
                nc.tensor.matmul(psS[:, t, :], KT[kvh][:, bass.ts(SLOT[j], 128)],
                                 QT[h][:, bass.ts(bi, 128)],
                                 start=True, stop=True)
            nv = len(bj)
            ex = p_exp.tile([128, 8, 128], BF16, tag='expb', name='expb')
            nc.scalar.activation(ex[:, 0:nv, :], psS[:, 0:nv, :], AF.Exp,
                                 scale=SCALE)
            for t, j in enumerate(bj):
                if j >= gi - 1:   # possible diagonal/overhang: apply mask
                    nc.vector.tensor_mul(ex[:, t, :], ex[:, t, :],
                                         dm_s[:, bi, j - (gi - 1), :])
            return ex

        def emit_ctx(u, ex, psC):
            h, bi, bj, last = u
            kvh = h // 2
            gi = BOUNDS[bi]
            for t, j in enumerate(bj):
                nc.tensor.matmul(psC[:, 0:129], ex[:, t, :],
                                 VA[kvh][:, SLOT[j], 0:129],
                                 start=(j == 0), stop=(j == gi))
            if last:
                rd = p_s.tile([128, 1], F32, tag='rd', name='rd')
                nc.vector.reciprocal(rd[:], psC[:, 128:129])
                cn = p_w.tile([128, 128], BF16, tag='cn', name='cn')
                nc.vector.tensor_scalar_mul(cn[:], psC[:, 0:128], rd[:])
                pst = ps_t.tile([128, 128], BF16, tag='psT', name='psT')
                nc.tensor.transpose(pst[:], cn[:], iden_s[:])
                nc.vector.tensor_copy(ctm[h][:, bass.ts(bi, 128)], pst[:])

        # delay-2 software pipeline: ctx of unit k emitted after scores of k+2
        from collections import deque
        pend = deque()   # (unit, ex, psC)
        cur_psC = None

        def drain_one():
            u2, ex2, psC2 = pend.popleft()
            emit_ctx(u2, ex2, psC2)
            if u2[3] and u2[1] == 7 and u2[0] in CTX_SPLIT:
                emit_ctx_gather(u2[0])

        for u in units:
            h, bi, bj, last = u
            if bj[0] == 0:
                cur_psC = ps_c.tile([128, 132], F32, tag='psC', name='psC')
            ex = emit_scores(u)
            pend.append((u, ex, cur_psC))
            if len(pend) > 2:
                drain_one()
        while pend:
            drain_one()

        # ---- o_proj (my od half columns); earlier-gathered chunks first ----
        attn_ctx.close()   # free psS/psC banks
        ps_o = ctx.enter_context(tc.tile_pool(name='psO', bufs=2, space='PSUM'))
        for bi in range(8):
            for nt in range(2):
                psO = ps_o.tile([128, 512], F32, tag='psO', name='psO')
                for i in range(16):
                    nc.tensor.matmul(psO[:], ctf[i][:, bass.ts(bi, 128)],
                                     wo_s[CTF_WO[i]][:, bass.ts(nt, 512)],
                                     start=(i == 0), stop=(i == 15))
                ob = p_out.tile([128, 512], F32, tag='outb', name='outb')
                nc.vector.tensor_copy(ob[:], psO[:])
                nc.sync.dma_start(out_e[bass.ts(bi, 128), bass.ts(nt, 512)], ob[:])

    split_multi_waits(nc)
    return nc


# ---------------------------------------------------------------------------
_NC_CACHE = None
_LAST_IN_MAPS = None


def _get_nc():
    global _NC_CACHE
    if _NC_CACHE is None:
        _NC_CACHE = build_kernel()
    return _NC_CACHE


def kernel(hidden_states, cos, sin, q_w, k_w, v_w, o_w, q_norm_w, k_norm_w):
    from concourse.bass_utils import run_bass_kernel_spmd

    hidden_states = np.asarray(hidden_states, np.float32)
    cos = np.asarray(cos, np.float32)
    sin = np.asarray(sin, np.float32)
    q_w = np.asarray(q_w, np.float32)
    k_w = np.asarray(k_w, np.float32)
    v_w = np.asarray(v_w, np.float32)
    o_w = np.asarray(o_w, np.float32)
    q_norm_w = np.asarray(q_norm_w, np.float32)
    k_norm_w = np.asarray(k_norm_w, np.float32)

    tri_np = np.triu(np.ones((128, 128), np.float32))  # [sj,si]: valid sj<=si
    iden_np = np.eye(128, dtype=BF)

    def rope_tabs(c, s_, w):
        # tables [rows, 4, 64]: A=c_lo*w_lo, B=s_lo*w_hi, C=c_lo*w_hi, D=s_lo*w_lo
        cl, sl = c[:, 0:64], s_[:, 0:64]
        wl, wh = w[0:64], w[64:128]
        return np.stack([cl * wl, sl * wh, cl * wh, sl * wl], axis=1).astype(BF)

    in_maps = []
    for c in range(8):
        # core layout: c = b*4 + hh*2 + sh, so the stripe pair (K/V exchange)
        # is same-SEngine adjacent (Mesh path) and the head pair is cross-SE
        b, hh, sh = c >> 2, (c >> 1) & 1, c & 1
        blks = MYBLKS[sh]
        rows = np.concatenate([np.arange(g * 128, (g + 1) * 128) for g in blks])
        hTq = np.ascontiguousarray(hidden_states[b][rows].T).astype(BF)
        qwT = np.ascontiguousarray(q_w[hh * 1024:(hh + 1) * 1024].T).astype(BF)
        kwT = np.ascontiguousarray(k_w[hh * 512:(hh + 1) * 512].T).astype(BF)
        vwT = np.ascontiguousarray(v_w[hh * 512:(hh + 1) * 512].T).astype(BF)
        owT = np.ascontiguousarray(o_w[hh * 1024:(hh + 1) * 1024].T).astype(BF)
        qtab = rope_tabs(cos[b][rows], sin[b][rows], q_norm_w)
        ktab = rope_tabs(cos[b][rows], sin[b][rows], k_norm_w)
        # diagonal masks dm[bi, t]: t=0 -> sj block gi-1, t=1 -> sj block gi
        # my true causal diagonal is at block g=blks[bi] (<= BOUNDS[bi]).
        dm_np = np.zeros((8, 2, 128, 128), np.float32)
        for bi in range(8):
            g, gb = blks[bi], BOUNDS[bi]
            for t, j in enumerate((gb - 1, gb)):
                if j < 0:
                    continue
                if j < g:
                    dm_np[bi, t] = 1.0
                elif j == g:
                    dm_np[bi, t] = tri_np
                # j > g: stays 0 (block fully masked)
        in_maps.append(dict(
            hTq=hTq, qwT=qwT, kwT=kwT, vwT=vwT, owT=owT,
            qtab=qtab, ktab=ktab, iden=iden_np, dm=dm_np.astype(BF)))

    global _LAST_IN_MAPS
    _LAST_IN_MAPS = in_maps
    nc = _get_nc()
    res = run_bass_kernel_spmd(nc, in_maps, core_ids=list(range(8)))

    out = np.zeros((B, S, HID), np.float32)
    for c in range(8):
        b, hh, sh = c >> 2, (c >> 1) & 1, c & 1
        o = res.results[c]['out']  # [1024, 1024]
        for bi, g in enumerate(MYBLKS[sh]):
            out[b, g * 128:(g + 1) * 128, hh * 1024:(hh + 1) * 1024] = \
                o[bi * 128:(bi + 1) * 128]
    return out


if __name__ == '__main__':
    sys.path.insert(0, '/root/problem')
    import reference
    inputs = {k: np.asarray(v) for k, v in reference.setup_inputs().items()}
    exp = np.asarray(reference.reference(**inputs))
    act = kernel(**inputs)
    err = np.abs(act - exp)
    rel = np.linalg.norm(act - exp) / np.linalg.norm(exp)
    print('Relative error:', rel, 'max abs err:', err.max())
